# revision 32
# baseline (speedup 1.0000x reference)
"""CrossNet (DCN cross layers) Trainium2 Bass kernel.

Reference computation (per batch row r, ORDER=3 layers):
    x_{i+1} = x0 * (x_i . w_i) + b_i + x_i
which collapses algebraically: x_i = a_i * x0 + v_i with
    v_i = sum_{j<i} b_j                 (constant vector)
    d_i = x0 . w_i                      (per-row scalar, all against x0)
    e_i = v_i . w_i                     (constant scalar)
    a_{i+1} = a_i * (1 + d_i) + e_i,    a_0 = 1
    out = a_3 * x0 + v_3
So the kernel only needs 3 dot products of each row against w_0..2, a tiny
scalar recurrence, and one final scale — one pass over x0 (memory bound).

Sharding: data-parallel over batch across 8 cores (2048 rows each). Each
core's shard is staged host-side in transposed layout x0T [DIM, ROWS] so the
dot-product contraction (over DIM) lands on the SBUF partition axis, where the
TensorEngine contracts natively. The per-row scales are selected+broadcast
back over partitions with a selector matmul, and the final scale is one
tensor_tensor multiply per tile. Output is produced transposed and
un-transposed on the host.

Matmuls use float32r (full-rate fp32 PE mode; plain fp32 runs as 2
half-speed passes). Tiles are k-major and batch-wide so every DMA moves
4KB-contiguous runs per partition.
"""

import numpy as np

BATCH, DIM, ORDER, NCORES = 16384, 1024, 3, 8
ROWS = BATCH // NCORES  # 2048 batch rows per core
P = 128                 # SBUF partitions
KCH = DIM // P          # 8 dim chunks of 128
# asymmetric batch shards: the big shard's compute hides under the load
# stream; the small final shard leaves only a short tail after loads end
SUBS = (1536, 512)
USE_F32R = True

_cache: dict = {}


def _build_nc(has_bias: bool):
    import concourse.bass as bass  # noqa: F401
    import concourse.tile as tile
    from concourse import bacc, mybir

    f32 = mybir.dt.float32
    fmm = mybir.dt.float32r if USE_F32R else f32

    nc = bacc.Bacc("TRN2", target_bir_lowering=False, debug=False)

    x0t = nc.dram_tensor("x0t", [DIM, ROWS], fmm, kind="ExternalInput")
    # packed constants, one DMA: cst[:, :KCH*ORDER] = wt (wt[p, 3k+i] =
    # W[i, 128k+p]); cst[0:3, KCH*ORDER:] = sel (sel[p, i*P+m] = (p == i))
    cst = nc.dram_tensor(
        "cst", [P, KCH * ORDER + ORDER * P], fmm, kind="ExternalInput"
    )
    if has_bias:
        # vb[p, k] = v3[128k+p]; ev[p, i] = e_{i+1} replicated over partitions
        vb = nc.dram_tensor("vb", [P, KCH], f32, kind="ExternalInput")
        ev = nc.dram_tensor("ev", [P, 2], f32, kind="ExternalInput")
    outt = nc.dram_tensor("outt", [DIM, ROWS], f32, kind="ExternalOutput")

    with tile.TileContext(nc) as tc:
        with (
            tc.tile_pool(name="consts", bufs=1) as consts,
            tc.tile_pool(name="xin", bufs=1) as xin_pool,
            tc.tile_pool(name="outp", bufs=8) as out_pool,
            tc.tile_pool(name="small", bufs=2) as small,
            tc.tile_pool(name="psum_d", bufs=1, space="PSUM") as psum_d,
            tc.tile_pool(name="psum_bc", bufs=1, space="PSUM") as psum_bc,
        ):
            cst_sb = consts.tile([P, KCH * ORDER + ORDER * P], fmm)
            nc.sync.dma_start(cst_sb[:], cst[:])
            wt_sb = cst_sb[:, :KCH * ORDER]
            sel_sb = cst_sb[0:ORDER, KCH * ORDER:]
            if has_bias:
                vb_sb = consts.tile([P, KCH], f32)
                nc.sync.dma_start(vb_sb[:], vb[:])
                ev_sb = consts.tile([P, 2], f32)
                nc.sync.dma_start(ev_sb[:], ev[:])

            # ---- all loads first: SP issues 16 loads back-to-back ----
            # (4-6KB/partition contiguous rows per DMA)
            xk = {}
            offs = [sum(SUBS[:i]) for i in range(len(SUBS))]
            for h, S in enumerate(SUBS):
                c0 = offs[h]
                for k in range(KCH):
                    t = xin_pool.tile([P, S], fmm, tag=f"x{k}{h}")
                    nc.sync.dma_start(t[:], x0t[k * P:(k + 1) * P, c0:c0 + S])
                    xk[(h, k)] = t

            for h, S in enumerate(SUBS):
                c0 = offs[h]
                NB = S // 512
                # ---- dot products: D[i, b] = sum_d x0[b, d] W[i, d] ----
                # k-outer so each tile's matmuls issue as its load lands;
                # after the last load only k=KCH-1's NB matmuls remain.
                d_ps = psum_d.tile([ORDER, S], f32, tag=f"dps{h}")
                for k in range(KCH):
                    for c in range(NB):
                        nc.tensor.matmul(
                            d_ps[:, c * 512:(c + 1) * 512],
                            wt_sb[:, ORDER * k:ORDER * (k + 1)],
                            xk[(h, k)][:, c * 512:(c + 1) * 512],
                            start=(k == 0),
                            stop=(k == KCH - 1),
                        )

                # d1 = 1 + D   (ACT, psum -> sbuf, rounds to f32r for PE)
                d1 = small.tile([ORDER, S], fmm, tag=f"d1{h}")
                nc.scalar.add(d1[:], d_ps[:], 1.0)

                # a chain per 512-col chunk: bc_i = select+broadcast row i
                a_sb = small.tile([P, S], f32, tag=f"asb{h}")
                for c in range(NB):
                    cs = slice(c * 512, (c + 1) * 512)
                    bcs = []
                    for i in range(ORDER):
                        bc = psum_bc.tile([P, 512], f32, tag=f"bc{i}")
                        nc.tensor.matmul(
                            bc[:],
                            sel_sb[:, i * P:(i + 1) * P],
                            d1[:, cs],
                        )
                        bcs.append(bc)
                    cp1 = small.tile([P, 512], f32, tag="cp1")
                    nc.scalar.copy(cp1[:], bcs[1][:])
                    if not has_bias:
                        t01 = small.tile([P, 512], f32, tag="t01")
                        nc.vector.tensor_mul(t01[:], bcs[0][:], cp1[:])
                        nc.vector.tensor_mul(a_sb[:, cs], t01[:], bcs[2][:])
                    else:
                        # a_{i+1} = a_i (1+d_i) + e_i
                        a2 = small.tile([P, 512], f32, tag="a2")
                        nc.vector.tensor_mul(a2[:], bcs[0][:], cp1[:])
                        nc.vector.tensor_scalar_add(a2[:], a2[:], ev_sb[:, 0:1])
                        a3 = small.tile([P, 512], f32, tag="a3")
                        nc.vector.tensor_mul(a3[:], a2[:], bcs[2][:])
                        nc.vector.tensor_scalar_add(
                            a_sb[:, cs], a3[:], ev_sb[:, 1:2]
                        )

                # ---- out_k = x0t_k * a  (+ v3), store per k-chunk ----
                for k in range(KCH):
                    ot = out_pool.tile([P, S], f32, tag="out")
                    if not has_bias:
                        nc.vector.tensor_mul(
                            ot[:], xk[(h, k)][:].bitcast(f32), a_sb[:]
                        )
                    else:
                        tmpk = small.tile([P, S], f32, tag="tmpk")
                        nc.vector.tensor_mul(
                            tmpk[:], xk[(h, k)][:].bitcast(f32), a_sb[:]
                        )
                        nc.scalar.add(ot[:], tmpk[:], vb_sb[:, k:k + 1])
                    # stores on SP after all loads in the SP stream
                    nc.sync.dma_start(
                        outt[k * P:(k + 1) * P, c0:c0 + S], ot[:]
                    )

    nc.compile()
    return nc


def _get_nc(has_bias: bool):
    key = ("nc", has_bias)
    if key not in _cache:
        _cache[key] = _build_nc(has_bias)
    return _cache[key]


def _host_aux(W: np.ndarray, B: np.ndarray):
    """cst/vb/ev staging buffers (tiny, host-side parameter folding)."""
    cst = np.zeros((P, KCH * ORDER + ORDER * P), dtype=np.float32)
    cst[:, :KCH * ORDER] = (
        W.T.reshape(KCH, P, ORDER).transpose(1, 0, 2).reshape(P, KCH * ORDER)
    )
    for i in range(ORDER):
        cst[i, KCH * ORDER + i * P:KCH * ORDER + (i + 1) * P] = 1.0
    v1 = B[0]
    v2 = B[0] + B[1]
    v3 = B[0] + B[1] + B[2]
    e1 = float(v1 @ W[1])
    e2 = float(v2 @ W[2])
    vb = np.ascontiguousarray(v3.reshape(KCH, P).T).astype(np.float32)
    ev = np.tile(np.array([[e1, e2]], dtype=np.float32), (P, 1))
    return cst, vb, ev


def kernel(x0: np.ndarray, W: np.ndarray, B: np.ndarray) -> np.ndarray:
    from concourse.bass_utils import run_bass_kernel_spmd

    x0 = np.asarray(x0, dtype=np.float32)
    W = np.asarray(W, dtype=np.float32)
    B = np.asarray(B, dtype=np.float32)

    has_bias = bool(np.any(B))
    nc = _get_nc(has_bias)
    cst, vb, ev = _host_aux(W, B)

    in_maps = []
    for c in range(NCORES):
        m = {
            "x0t": np.ascontiguousarray(x0[c * ROWS:(c + 1) * ROWS].T),
            "cst": cst,
        }
        if has_bias:
            m["vb"] = vb
            m["ev"] = ev
        in_maps.append(m)

    res = run_bass_kernel_spmd(nc, in_maps, list(range(NCORES)))

    out = np.empty((BATCH, DIM), dtype=np.float32)
    for c in range(NCORES):
        out[c * ROWS:(c + 1) * ROWS] = res.results[c]["outt"].T
    return out


# revision 34
# speedup vs baseline: 1.2083x; 1.2083x over previous
"""CrossNet (DCN cross layers) Trainium2 Bass kernel.

Reference computation (per batch row r, ORDER=3 layers):
    x_{i+1} = x0 * (x_i . w_i) + b_i + x_i
which collapses algebraically: x_i = a_i * x0 + v_i with
    v_i = sum_{j<i} b_j                 (constant vector)
    d_i = x0 . w_i                      (per-row scalar, all against x0)
    e_i = v_i . w_i                     (constant scalar)
    a_{i+1} = a_i * (1 + d_i) + e_i,    a_0 = 1
    out = a_3 * x0 + v_3
So the kernel only needs 3 dot products of each row against w_0..2, a tiny
scalar recurrence, and one final scale — one pass over x0 (memory bound).

Sharding: data-parallel over batch across 8 cores (2048 rows each). Each
core's shard is staged host-side in transposed layout x0T [DIM, ROWS] so the
dot-product contraction (over DIM) lands on the SBUF partition axis, where the
TensorEngine contracts natively. The per-row scales are selected+broadcast
back over partitions with a selector matmul, and the final scale is one
tensor_tensor multiply per tile. Output is produced transposed and
un-transposed on the host.

Matmuls use float32r (full-rate fp32 PE mode; plain fp32 runs as 2
half-speed passes). Tiles are k-major and batch-wide so every DMA moves
4KB-contiguous runs per partition.
"""

import numpy as np

BATCH, DIM, ORDER, NCORES = 16384, 1024, 3, 8
ROWS = BATCH // NCORES  # 2048 batch rows per core
P = 128                 # SBUF partitions
KCH = DIM // P          # 8 dim chunks of 128
SUBS = (1024, 1024)     # batch shards per core (pipeline granularity)
USE_F32R = True

_cache: dict = {}


def _build_nc(has_bias: bool):
    import concourse.bass as bass  # noqa: F401
    import concourse.tile as tile
    from concourse import bacc, mybir

    f32 = mybir.dt.float32
    fmm = mybir.dt.float32r if USE_F32R else f32

    nc = bacc.Bacc("TRN2", target_bir_lowering=False, debug=False)

    x0t = nc.dram_tensor("x0t", [DIM, ROWS], fmm, kind="ExternalInput")
    # packed constants, one DMA: cst[:, :KCH*ORDER] = wt (wt[p, 3k+i] =
    # W[i, 128k+p]); cst[0:3, KCH*ORDER:] = sel (sel[p, i*P+m] = (p == i))
    cst = nc.dram_tensor(
        "cst", [P, KCH * ORDER + ORDER * P], fmm, kind="ExternalInput"
    )
    if has_bias:
        # vb[p, k] = v3[128k+p]; ev[p, i] = e_{i+1} replicated over partitions
        vb = nc.dram_tensor("vb", [P, KCH], f32, kind="ExternalInput")
        ev = nc.dram_tensor("ev", [P, 2], f32, kind="ExternalInput")
    outt = nc.dram_tensor("outt", [DIM, ROWS], f32, kind="ExternalOutput")

    with tile.TileContext(nc) as tc:
        with (
            tc.tile_pool(name="consts", bufs=1) as consts,
            tc.tile_pool(name="xin", bufs=1) as xin_pool,
            tc.tile_pool(name="outp", bufs=8) as out_pool,
            tc.tile_pool(name="small", bufs=2) as small,
            tc.tile_pool(name="psum_d", bufs=1, space="PSUM") as psum_d,
            tc.tile_pool(name="psum_bc", bufs=1, space="PSUM") as psum_bc,
        ):
            cst_sb = consts.tile([P, KCH * ORDER + ORDER * P], fmm)
            nc.sync.dma_start(cst_sb[:], cst[:])
            wt_sb = cst_sb[:, :KCH * ORDER]
            sel_sb = cst_sb[0:ORDER, KCH * ORDER:]
            if has_bias:
                vb_sb = consts.tile([P, KCH], f32)
                nc.sync.dma_start(vb_sb[:], vb[:])
                ev_sb = consts.tile([P, 2], f32)
                nc.sync.dma_start(ev_sb[:], ev[:])

            # ---- all loads first: SP issues 16 loads back-to-back ----
            # (4-6KB/partition contiguous rows per DMA)
            xk = {}
            offs = [sum(SUBS[:i]) for i in range(len(SUBS))]
            for h, S in enumerate(SUBS):
                c0 = offs[h]
                for k in range(KCH):
                    t = xin_pool.tile([P, S], fmm, tag=f"x{k}{h}")
                    nc.sync.dma_start(t[:], x0t[k * P:(k + 1) * P, c0:c0 + S])
                    xk[(h, k)] = t

            d1s, a_sbs = {}, {}

            def emit_D(h):
                # k-outer so each tile's matmuls issue as its load lands;
                # after the last load only k=KCH-1's NB matmuls remain.
                S = SUBS[h]
                d_ps = psum_d.tile([ORDER, S], f32, tag=f"dps{h}")
                for k in range(KCH):
                    for c in range(S // 512):
                        nc.tensor.matmul(
                            d_ps[:, c * 512:(c + 1) * 512],
                            wt_sb[:, ORDER * k:ORDER * (k + 1)],
                            xk[(h, k)][:, c * 512:(c + 1) * 512],
                            start=(k == 0),
                            stop=(k == KCH - 1),
                        )
                # d1 = 1 + D   (ACT, psum -> sbuf, rounds to f32r for PE)
                d1 = small.tile([ORDER, S], fmm, tag=f"d1{h}")
                nc.scalar.add(d1[:], d_ps[:], 1.0)
                d1s[h] = d1

            def emit_ladder(h):
                # a chain per 512-col chunk: bc_i = select+broadcast row i
                S = SUBS[h]
                d1 = d1s[h]
                a_sb = small.tile([P, S], f32, tag=f"asb{h}")
                for c in range(S // 512):
                    cs = slice(c * 512, (c + 1) * 512)
                    bcs = []
                    for i in range(ORDER):
                        bc = psum_bc.tile([P, 512], f32, tag=f"bc{i}")
                        nc.tensor.matmul(
                            bc[:],
                            sel_sb[:, i * P:(i + 1) * P],
                            d1[:, cs],
                        )
                        bcs.append(bc)
                    cp1 = small.tile([P, 512], f32, tag="cp1")
                    nc.scalar.copy(cp1[:], bcs[1][:])
                    if not has_bias:
                        t01 = small.tile([P, 512], f32, tag="t01")
                        nc.vector.tensor_mul(t01[:], bcs[0][:], cp1[:])
                        nc.vector.tensor_mul(a_sb[:, cs], t01[:], bcs[2][:])
                    else:
                        # a_{i+1} = a_i (1+d_i) + e_i
                        a2 = small.tile([P, 512], f32, tag="a2")
                        nc.vector.tensor_mul(a2[:], bcs[0][:], cp1[:])
                        nc.vector.tensor_scalar_add(a2[:], a2[:], ev_sb[:, 0:1])
                        a3 = small.tile([P, 512], f32, tag="a3")
                        nc.vector.tensor_mul(a3[:], a2[:], bcs[2][:])
                        nc.vector.tensor_scalar_add(
                            a_sb[:, cs], a3[:], ev_sb[:, 1:2]
                        )
                a_sbs[h] = a_sb

            def emit_muls_stores(h):
                # out_k = x0t_k * a  (+ v3); store per k-chunk on the ACT
                # HWDGE ring (parallel to the SP ring carrying the loads)
                S, c0 = SUBS[h], offs[h]
                a_sb = a_sbs[h]
                for k in range(KCH):
                    ot = out_pool.tile([P, S], f32, tag="out")
                    if not has_bias:
                        nc.vector.tensor_mul(
                            ot[:], xk[(h, k)][:].bitcast(f32), a_sb[:]
                        )
                    else:
                        tmpk = small.tile([P, S], f32, tag="tmpk")
                        nc.vector.tensor_mul(
                            tmpk[:], xk[(h, k)][:].bitcast(f32), a_sb[:]
                        )
                        nc.scalar.add(ot[:], tmpk[:], vb_sb[:, k:k + 1])
                    nc.scalar.dma_start(
                        outt[k * P:(k + 1) * P, c0:c0 + S], ot[:]
                    )

            # Emission order = per-engine program order. d1(h1) is emitted
            # before stores(h0) so the ACT stream doesn't park h1's ladder
            # behind store issues; h1's ladder TTs come after muls(h0) so
            # the DVE stream doesn't park muls(h0) behind them.
            emit_D(0)
            emit_ladder(0)
            emit_D(1)
            emit_muls_stores(0)
            emit_ladder(1)
            emit_muls_stores(1)

    nc.compile()
    return nc


def _get_nc(has_bias: bool):
    key = ("nc", has_bias)
    if key not in _cache:
        _cache[key] = _build_nc(has_bias)
    return _cache[key]


def _host_aux(W: np.ndarray, B: np.ndarray):
    """cst/vb/ev staging buffers (tiny, host-side parameter folding)."""
    cst = np.zeros((P, KCH * ORDER + ORDER * P), dtype=np.float32)
    cst[:, :KCH * ORDER] = (
        W.T.reshape(KCH, P, ORDER).transpose(1, 0, 2).reshape(P, KCH * ORDER)
    )
    for i in range(ORDER):
        cst[i, KCH * ORDER + i * P:KCH * ORDER + (i + 1) * P] = 1.0
    v1 = B[0]
    v2 = B[0] + B[1]
    v3 = B[0] + B[1] + B[2]
    e1 = float(v1 @ W[1])
    e2 = float(v2 @ W[2])
    vb = np.ascontiguousarray(v3.reshape(KCH, P).T).astype(np.float32)
    ev = np.tile(np.array([[e1, e2]], dtype=np.float32), (P, 1))
    return cst, vb, ev


def kernel(x0: np.ndarray, W: np.ndarray, B: np.ndarray) -> np.ndarray:
    from concourse.bass_utils import run_bass_kernel_spmd

    x0 = np.asarray(x0, dtype=np.float32)
    W = np.asarray(W, dtype=np.float32)
    B = np.asarray(B, dtype=np.float32)

    has_bias = bool(np.any(B))
    nc = _get_nc(has_bias)
    cst, vb, ev = _host_aux(W, B)

    in_maps = []
    for c in range(NCORES):
        m = {
            "x0t": np.ascontiguousarray(x0[c * ROWS:(c + 1) * ROWS].T),
            "cst": cst,
        }
        if has_bias:
            m["vb"] = vb
            m["ev"] = ev
        in_maps.append(m)

    res = run_bass_kernel_spmd(nc, in_maps, list(range(NCORES)))

    out = np.empty((BATCH, DIM), dtype=np.float32)
    for c in range(NCORES):
        out[c * ROWS:(c + 1) * ROWS] = res.results[c]["outt"].T
    return out


# revision 36
# speedup vs baseline: 1.2490x; 1.0337x over previous
"""CrossNet (DCN cross layers) Trainium2 Bass kernel.

Reference computation (per batch row r, ORDER=3 layers):
    x_{i+1} = x0 * (x_i . w_i) + b_i + x_i
which collapses algebraically: x_i = a_i * x0 + v_i with
    v_i = sum_{j<i} b_j                 (constant vector)
    d_i = x0 . w_i                      (per-row scalar, all against x0)
    e_i = v_i . w_i                     (constant scalar)
    a_{i+1} = a_i * (1 + d_i) + e_i,    a_0 = 1
    out = a_3 * x0 + v_3
So the kernel only needs 3 dot products of each row against w_0..2, a tiny
scalar recurrence, and one final scale — one pass over x0 (memory bound).

Sharding: data-parallel over batch across 8 cores (2048 rows each). Each
core's shard is staged host-side in transposed layout x0T [DIM, ROWS] so the
dot-product contraction (over DIM) lands on the SBUF partition axis, where the
TensorEngine contracts natively. The per-row scales are selected+broadcast
back over partitions with a selector matmul, and the final scale is one
tensor_tensor multiply per tile. Output is produced transposed and
un-transposed on the host.

Matmuls use float32r (full-rate fp32 PE mode; plain fp32 runs as 2
half-speed passes). Tiles are k-major and batch-wide so every DMA moves
4KB-contiguous runs per partition.
"""

import numpy as np

BATCH, DIM, ORDER, NCORES = 16384, 1024, 3, 8
ROWS = BATCH // NCORES  # 2048 batch rows per core
P = 128                 # SBUF partitions
KCH = DIM // P          # 8 dim chunks of 128
SUBS = (1024, 1024)     # batch shards per core (pipeline granularity)
USE_F32R = True

_cache: dict = {}


def _build_nc(has_bias: bool):
    import concourse.bass as bass  # noqa: F401
    import concourse.tile as tile
    from concourse import bacc, mybir

    f32 = mybir.dt.float32
    fmm = mybir.dt.float32r if USE_F32R else f32

    nc = bacc.Bacc("TRN2", target_bir_lowering=False, debug=False)

    x0t = nc.dram_tensor("x0t", [DIM, ROWS], fmm, kind="ExternalInput")
    # packed constants, one DMA: cst[:, :KCH*ORDER] = wt (wt[p, 3k+i] =
    # W[i, 128k+p]); cst[0:3, KCH*ORDER:] = sel (sel[p, i*P+m] = (p == i))
    cst = nc.dram_tensor(
        "cst", [P, KCH * ORDER + ORDER * P], fmm, kind="ExternalInput"
    )
    if has_bias:
        # vb[p, k] = v3[128k+p]; ev[p, i] = e_{i+1} replicated over partitions
        vb = nc.dram_tensor("vb", [P, KCH], f32, kind="ExternalInput")
        ev = nc.dram_tensor("ev", [P, 2], f32, kind="ExternalInput")
    outt = nc.dram_tensor("outt", [DIM, ROWS], f32, kind="ExternalOutput")

    with tile.TileContext(nc) as tc:
        with (
            tc.tile_pool(name="consts", bufs=1) as consts,
            tc.tile_pool(name="xin", bufs=1) as xin_pool,
            tc.tile_pool(name="outp", bufs=8) as out_pool,
            tc.tile_pool(name="small", bufs=2) as small,
            tc.tile_pool(name="psum_d", bufs=1, space="PSUM") as psum_d,
            tc.tile_pool(name="psum_bc", bufs=1, space="PSUM") as psum_bc,
        ):
            cst_sb = consts.tile([P, KCH * ORDER + ORDER * P], fmm)
            nc.sync.dma_start(cst_sb[:], cst[:])
            wt_sb = cst_sb[:, :KCH * ORDER]
            sel_sb = cst_sb[0:ORDER, KCH * ORDER:]
            if has_bias:
                vb_sb = consts.tile([P, KCH], f32)
                nc.sync.dma_start(vb_sb[:], vb[:])
                ev_sb = consts.tile([P, 2], f32)
                nc.sync.dma_start(ev_sb[:], ev[:])

            # ---- all loads first: SP issues 16 loads back-to-back ----
            # (4KB/partition contiguous rows per DMA). The SDMA engines
            # round-robin across queued DMAs, so without the explicit dep
            # edges below the h0/h1 loads would complete interleaved and
            # h0's compute couldn't start until nearly all loads landed.
            from concourse.tile_rust import add_dep_helper

            xk = {}
            load_insts = {}
            offs = [sum(SUBS[:i]) for i in range(len(SUBS))]
            for h, S in enumerate(SUBS):
                c0 = offs[h]
                for k in range(KCH):
                    t = xin_pool.tile([P, S], fmm, tag=f"x{k}{h}")
                    li = nc.sync.dma_start(
                        t[:], x0t[k * P:(k + 1) * P, c0:c0 + S]
                    )
                    if h > 0:
                        add_dep_helper(
                            li.ins,
                            load_insts[(h - 1, k)],
                            reason="phase loads: h waits h-1 data",
                        )
                    load_insts[(h, k)] = li.ins
                    xk[(h, k)] = t

            d1s, a_sbs = {}, {}

            def emit_D(h):
                # k-outer so each tile's matmuls issue as its load lands;
                # after the last load only k=KCH-1's NB matmuls remain.
                S = SUBS[h]
                d_ps = psum_d.tile([ORDER, S], f32, tag=f"dps{h}")
                for k in range(KCH):
                    for c in range(S // 512):
                        nc.tensor.matmul(
                            d_ps[:, c * 512:(c + 1) * 512],
                            wt_sb[:, ORDER * k:ORDER * (k + 1)],
                            xk[(h, k)][:, c * 512:(c + 1) * 512],
                            start=(k == 0),
                            stop=(k == KCH - 1),
                        )
                # d1 = 1 + D   (ACT, psum -> sbuf, rounds to f32r for PE)
                d1 = small.tile([ORDER, S], fmm, tag=f"d1{h}")
                nc.scalar.add(d1[:], d_ps[:], 1.0)
                d1s[h] = d1

            def emit_ladder(h):
                # a chain per 512-col chunk: bc_i = select+broadcast row i
                S = SUBS[h]
                d1 = d1s[h]
                a_sb = small.tile([P, S], f32, tag=f"asb{h}")
                for c in range(S // 512):
                    cs = slice(c * 512, (c + 1) * 512)
                    bcs = []
                    for i in range(ORDER):
                        bc = psum_bc.tile([P, 512], f32, tag=f"bc{i}")
                        nc.tensor.matmul(
                            bc[:],
                            sel_sb[:, i * P:(i + 1) * P],
                            d1[:, cs],
                        )
                        bcs.append(bc)
                    cp1 = small.tile([P, 512], f32, tag="cp1")
                    nc.scalar.copy(cp1[:], bcs[1][:])
                    if not has_bias:
                        t01 = small.tile([P, 512], f32, tag="t01")
                        nc.vector.tensor_mul(t01[:], bcs[0][:], cp1[:])
                        nc.vector.tensor_mul(a_sb[:, cs], t01[:], bcs[2][:])
                    else:
                        # a_{i+1} = a_i (1+d_i) + e_i
                        a2 = small.tile([P, 512], f32, tag="a2")
                        nc.vector.tensor_mul(a2[:], bcs[0][:], cp1[:])
                        nc.vector.tensor_scalar_add(a2[:], a2[:], ev_sb[:, 0:1])
                        a3 = small.tile([P, 512], f32, tag="a3")
                        nc.vector.tensor_mul(a3[:], a2[:], bcs[2][:])
                        nc.vector.tensor_scalar_add(
                            a_sb[:, cs], a3[:], ev_sb[:, 1:2]
                        )
                a_sbs[h] = a_sb

            def emit_muls_stores(h):
                # out_k = x0t_k * a  (+ v3); store per k-chunk on the ACT
                # HWDGE ring (parallel to the SP ring carrying the loads)
                S, c0 = SUBS[h], offs[h]
                a_sb = a_sbs[h]
                for k in range(KCH):
                    ot = out_pool.tile([P, S], f32, tag="out")
                    if not has_bias:
                        nc.vector.tensor_mul(
                            ot[:], xk[(h, k)][:].bitcast(f32), a_sb[:]
                        )
                    else:
                        tmpk = small.tile([P, S], f32, tag="tmpk")
                        nc.vector.tensor_mul(
                            tmpk[:], xk[(h, k)][:].bitcast(f32), a_sb[:]
                        )
                        nc.scalar.add(ot[:], tmpk[:], vb_sb[:, k:k + 1])
                    nc.scalar.dma_start(
                        outt[k * P:(k + 1) * P, c0:c0 + S], ot[:]
                    )

            # Emission order = per-engine program order. d1(h1) is emitted
            # before stores(h0) so the ACT stream doesn't park h1's ladder
            # behind store issues; h1's ladder TTs come after muls(h0) so
            # the DVE stream doesn't park muls(h0) behind them.
            emit_D(0)
            emit_ladder(0)
            emit_D(1)
            emit_muls_stores(0)
            emit_ladder(1)
            emit_muls_stores(1)

    nc.compile()
    return nc


def _get_nc(has_bias: bool):
    key = ("nc", has_bias)
    if key not in _cache:
        _cache[key] = _build_nc(has_bias)
    return _cache[key]


def _host_aux(W: np.ndarray, B: np.ndarray):
    """cst/vb/ev staging buffers (tiny, host-side parameter folding)."""
    cst = np.zeros((P, KCH * ORDER + ORDER * P), dtype=np.float32)
    cst[:, :KCH * ORDER] = (
        W.T.reshape(KCH, P, ORDER).transpose(1, 0, 2).reshape(P, KCH * ORDER)
    )
    for i in range(ORDER):
        cst[i, KCH * ORDER + i * P:KCH * ORDER + (i + 1) * P] = 1.0
    v1 = B[0]
    v2 = B[0] + B[1]
    v3 = B[0] + B[1] + B[2]
    e1 = float(v1 @ W[1])
    e2 = float(v2 @ W[2])
    vb = np.ascontiguousarray(v3.reshape(KCH, P).T).astype(np.float32)
    ev = np.tile(np.array([[e1, e2]], dtype=np.float32), (P, 1))
    return cst, vb, ev


def kernel(x0: np.ndarray, W: np.ndarray, B: np.ndarray) -> np.ndarray:
    from concourse.bass_utils import run_bass_kernel_spmd

    x0 = np.asarray(x0, dtype=np.float32)
    W = np.asarray(W, dtype=np.float32)
    B = np.asarray(B, dtype=np.float32)

    has_bias = bool(np.any(B))
    nc = _get_nc(has_bias)
    cst, vb, ev = _host_aux(W, B)

    in_maps = []
    for c in range(NCORES):
        m = {
            "x0t": np.ascontiguousarray(x0[c * ROWS:(c + 1) * ROWS].T),
            "cst": cst,
        }
        if has_bias:
            m["vb"] = vb
            m["ev"] = ev
        in_maps.append(m)

    res = run_bass_kernel_spmd(nc, in_maps, list(range(NCORES)))

    out = np.empty((BATCH, DIM), dtype=np.float32)
    for c in range(NCORES):
        out[c * ROWS:(c + 1) * ROWS] = res.results[c]["outt"].T
    return out
